# revision 1
# baseline (speedup 1.0000x reference)
"""Performer (FAVOR+) linear attention on 8 TRN2 NeuronCores.

Sharding: core c handles batch b=c//4 and head group g=c%4 (4 of 16 heads).
Each core computes q/k/v projections for its heads from its batch's x,
runs the per-head softmax-kernel + linear-attention chain, and produces a
partial output projection y_c = o_heads @ Wo_slice.T.  Host sums the 4
partials per batch and adds the bias.

Matmuls use float32r (fp32 with 12 low mantissa bits dropped; 4x faster on
the PE at free-dim>=256).  End-to-end absmax error vs the fp32 reference is
~8e-4 relative to output scale (measured via numpy simulation of the
rounding).
"""
import sys
sys.path.insert(0, '/opt/trn_rl_repo')

import numpy as np
import concourse.bass as bass
import concourse.bacc as bacc
import concourse.tile as tile
from concourse import mybir
from concourse.bass_utils import run_bass_kernel_spmd

F32 = mybir.dt.float32
F32R = mybir.dt.float32r
AX = mybir.AxisListType.X
AF = mybir.ActivationFunctionType

B, N, D = 2, 4096, 1024
H, DH, M = 16, 64, 266            # heads, dim_head, nb_features
HPC = 4                           # heads per core
EPS = 1e-4
CNORM = DH ** -0.25               # data normalizer
RATIO = M ** -0.5
LNR = float(np.log(RATIO))
NT = N // 128                     # 32 n-tiles
# m-chunks over the 267-wide (ones col at 0, then 266 m cols) kp/qp tiles
MCH = [(0, 128), (128, 128), (256, 11)]  # (off, width)
LIMIT = "all"  # debug: p1 | k1 | k2 | q | all


def build():
    nc = bacc.Bacc("TRN2", target_bir_lowering=False, debug=False)

    xT = nc.dram_tensor("xT", [D, N], F32, kind="ExternalInput")
    wqT = nc.dram_tensor("wqT", [D, 256], F32, kind="ExternalInput")
    wkT = nc.dram_tensor("wkT", [D, 256], F32, kind="ExternalInput")
    wvT = nc.dram_tensor("wvT", [D, 256], F32, kind="ExternalInput")
    woP = nc.dram_tensor("woP", [128, 2048], F32, kind="ExternalInput")
    projc = nc.dram_tensor("projc", [DH, M], F32, kind="ExternalInput")
    ident = nc.dram_tensor("ident", [128, 128], F32, kind="ExternalInput")
    y = nc.dram_tensor("y", [N, D], F32, kind="ExternalOutput")
    qkv_d = nc.dram_tensor("qkv_scr", [3, HPC, N, DH], F32, kind="Internal")

    with tile.TileContext(nc) as tc:
        ctx_mgr = tc.tile_pool(name="const", bufs=1)
        with ctx_mgr as cpool, \
             tc.tile_pool(name="stage", bufs=1) as stg, \
             tc.tile_pool(name="stream", bufs=3) as strm, \
             tc.tile_pool(name="big", bufs=2) as big, \
             tc.tile_pool(name="small", bufs=4) as sml, \
             tc.tile_pool(name="ot", bufs=1) as otp_pool, \
             tc.tile_pool(name="ps", bufs=2, space="PSUM") as ps, \
             tc.tile_pool(name="psc", bufs=1, space="PSUM") as psc, \
             tc.tile_pool(name="pst", bufs=1, space="PSUM") as pst:

            # ---- constants / weights ----
            wq_r = cpool.tile([128, 8, 256], F32R, tag="wq")
            wk_r = cpool.tile([128, 8, 256], F32R, tag="wk")
            wv_r = cpool.tile([128, 8, 256], F32R, tag="wv")
            wo_r = cpool.tile([128, 2048], F32R, tag="wo")
            projc_r = cpool.tile([DH, M], F32R, tag="pj")
            ident_f = cpool.tile([128, 128], F32, tag="idf")
            ident_r = cpool.tile([128, 128], F32R, tag="idr")
            ones1_f = cpool.tile([1, 128], F32, tag="o1f")
            ones1_r = cpool.tile([1, 128], F32R, tag="o1r")
            ones128 = cpool.tile([128, 1], F32R, tag="o128")

            for dst, src in ((wq_r, wqT), (wk_r, wkT), (wv_r, wvT)):
                st = stg.tile([128, 8, 256], F32, tag="wstage")
                nc.sync.dma_start(st[:], src.ap().rearrange("(c p) n -> p c n", p=128))
                nc.vector.tensor_copy(dst[:], st[:])
            st = stg.tile([128, 2048], F32, tag="wstage")
            nc.sync.dma_start(st[:], woP.ap())
            nc.vector.tensor_copy(wo_r[:], st[:])
            st = stg.tile([DH, M], F32, tag="pstage")
            nc.sync.dma_start(st[:], projc.ap())
            nc.vector.tensor_copy(projc_r[:], st[:])
            nc.sync.dma_start(ident_f[:], ident.ap())
            nc.scalar.copy(ident_r[:], ident_f[:])
            nc.vector.memset(ones1_f[:], 1.0)
            nc.scalar.copy(ones1_r[:], ones1_f[:])
            nc.scalar.activation(ones128[:], ident_f[:, 0:1], AF.Identity,
                                 bias=1.0, scale=0.0)

            # ---- phase 1: QKV projections, spilled to DRAM scratch ----
            for j in range(NT):
                xt = strm.tile([128, 8, 128], F32, tag="xt", bufs=2)
                nc.sync.dma_start(
                    xt[:], xT.ap().rearrange("(c p) n -> p c n", p=128)[:, :, j*128:(j+1)*128])
                xt_r = strm.tile([128, 8, 128], F32R, tag="xtr", bufs=2)
                nc.vector.tensor_copy(xt_r[:], xt[:])
                for ti, w_r in ((0, wq_r), (1, wk_r), (2, wv_r)):
                    acc = ps.tile([128, 256], F32, tag="b256")
                    for dchunk in range(8):
                        nc.tensor.matmul(acc[:], xt_r[:, dchunk, :], w_r[:, dchunk, :],
                                         start=(dchunk == 0), stop=(dchunk == 7))
                    sb = strm.tile([128, 4, DH], F32, tag=f"qkv{ti}", bufs=2)
                    nc.vector.tensor_copy(sb[:], acc[:].rearrange("p (h e) -> p h e", e=DH))
                    nc.sync.dma_start(
                        qkv_d.ap()[ti].rearrange("h (j p) e -> j p h e", p=128)[j], sb[:])

            # ---- per-head chain ----
            nheads = 0 if LIMIT == "p1" else (1 if LIMIT in ("k1", "k2", "q") else HPC)  # "heads": 4 heads, no P3
            for h in range(nheads):
                kb = big.tile([128, NT, DH], F32, tag="hb", bufs=3, name="kb")
                nc.sync.dma_start(
                    kb[:], qkv_d.ap()[1, h].rearrange("(j p) e -> p j e", p=128))
                dashk = big.tile([128, NT, M], F32, tag="dashk", bufs=1)
                rmaxb = sml.tile([128, NT], F32, tag="rmaxb", bufs=2)
                diagk = sml.tile([128, NT], F32, tag="diagk", bufs=2)

                # K1: dash_k tiles, running stats
                for j in range(NT):
                    sqj = strm.tile([128, DH], F32, tag="sqj")
                    nc.scalar.activation(sqj[:], kb[:, j, :], AF.Square,
                                         accum_out=diagk[:, j:j+1])
                    ktp = pst.tile([DH, 128], F32, tag="tp64")
                    nc.tensor.transpose(ktp[:], kb[:, j, :], ident_f[:])
                    kt = strm.tile([DH, 128], F32R, tag="kt")
                    nc.scalar.copy(kt[:], ktp[:])
                    dash = ps.tile([128, M], F32, tag="b256")
                    nc.tensor.matmul(dash[:], kt[:], projc_r[:], start=True, stop=True)
                    nc.vector.tensor_copy(dashk[:, j, :], dash[:])
                    if j % 4 == 3:
                        nc.vector.reduce_max(rmaxb[:, j-3:j+1],
                                             dashk[:, j-3:j+1, :], axis=AX)

                if LIMIT == "k1":
                    continue
                # global max -> per-partition broadcast mkb
                gmax = sml.tile([128, 1], F32, tag="gmax")
                nc.vector.reduce_max(gmax[:], rmaxb[:], axis=AX)
                gm_ps = pst.tile([1, 128], F32, tag="tp64")
                nc.tensor.transpose(gm_ps[:], gmax[:], ident_f[:])
                gmrow = sml.tile([1, 128], F32, tag="gmrow")
                nc.vector.tensor_copy(gmrow[:], gm_ps[:])
                mk = sml.tile([1, 1], F32, tag="mk")
                nc.vector.reduce_max(mk[:], gmrow[:], axis=AX)
                mk_ps = pst.tile([128, 1], F32, tag="tp64")
                nc.tensor.matmul(mk_ps[:], ones1_f[:], mk[:], start=True, stop=True)
                mkl = sml.tile([128, 1], F32, tag="mkl")
                # mkl = lnr - mk
                nc.vector.tensor_scalar(mkl[:], mk_ps[:], -1.0, LNR,
                                        op0=mybir.AluOpType.mult, op1=mybir.AluOpType.add)

                # batched per-tile exp bias: -0.5c^2*diag + (lnr - mk)
                biaskb = sml.tile([128, NT], F32, tag="biaskb", bufs=2)
                nc.vector.tensor_scalar(biaskb[:], diagk[:],
                                        -0.5 * CNORM * CNORM, mkl[:],
                                        op0=mybir.AluOpType.mult,
                                        op1=mybir.AluOpType.add)

                # K2: kp = exp(dash - 0.5c^2*diag - mk + lnr), context accumulation
                vb = big.tile([128, NT, DH], F32, tag="hb", bufs=3, name="vb")
                nc.sync.dma_start(
                    vb[:], qkv_d.ap()[2, h].rearrange("(j p) e -> p j e", p=128))
                vxb = big.tile([128, NT, 66], F32R, tag="vxb", bufs=2)
                nc.scalar.copy(vxb[:, :, 0:DH], vb[:])
                nc.scalar.activation(vxb[:, :, DH:66], vb[:, :, 0:2], AF.Identity,
                                     bias=1.0, scale=0.0)
                ctx_ps = [psc.tile([128, 66], F32, tag=f"ctx{mc}", name=f"ctxp{mc}") for mc in range(2)]
                ctx_ps.append(psc.tile([11, 66], F32, tag="ctx2", name="ctxp2"))
                colsum_ps = pst.tile([1, 66], F32, tag="oe", name="colsum_ps")
                for j in range(NT):
                    # kp col 0 is a ones column (for k_cumsum); cols 1..266 = kp
                    kp = strm.tile([128, M + 1], F32R, tag="kp")
                    nc.scalar.activation(kp[:, 1:M+1], dashk[:, j, :], AF.Exp,
                                         bias=biaskb[:, j:j+1], scale=1.0)
                    nc.scalar.activation(kp[:, 0:1], kp[:, 1:2], AF.Identity,
                                         bias=1.0, scale=0.0)
                    for mc, (off, w) in enumerate(MCH):
                        nc.tensor.matmul(ctx_ps[mc][:], kp[:, off:off+w],
                                         vxb[:, j, :], start=(j == 0), stop=False)
                    # colsum_v accumulator (own group so it can be read while
                    # the ctx groups are still open)
                    nc.tensor.matmul(colsum_ps[:], kp[:, 0:1], vxb[:, j, :],
                                     start=(j == 0), stop=(j == NT - 1))

                # eps-correction: ctx += ratio*eps * ones x colsum_v.  Row 0 of
                # chunk0 (the kp-ones-column product) gets corrupted by the
                # correction; it is overwritten with the qp-side eps row below.
                colsum = sml.tile([1, 66], F32R, tag="colsum")
                nc.scalar.mul(colsum[:], colsum_ps[:], RATIO * EPS)
                for mc, (off, w) in enumerate(MCH):
                    nc.tensor.matmul(ctx_ps[mc][:], ones1_r[:, 0:w],
                                     colsum[:], start=False, stop=True)
                ctx_s = [big.tile([128, 66], F32R, tag=f"ctxs{mc}", name=f"ctxs{mc}") for mc in range(2)]
                ctx_s.append(big.tile([11, 66], F32R, tag="ctxs2", name="ctxs2"))
                for mc in range(3):
                    nc.scalar.copy(ctx_s[mc][:], ctx_ps[mc][:])
                # sum of corrected ctx over all rows, minus the (corrected)
                # row 0, = sum over real m rows
                smc_ps = pst.tile([1, 66], F32, tag="tp64")
                for mc, (off, w) in enumerate(MCH):
                    nc.tensor.matmul(smc_ps[:], ones128[0:w, :], ctx_s[mc][:],
                                     start=(mc == 0), stop=(mc == 2))
                smc_s = sml.tile([1, 66], F32, tag="smcs")
                nc.vector.tensor_copy(smc_s[:], smc_ps[:])
                eps_t = sml.tile([1, 66], F32, tag="epst")
                nc.vector.tensor_sub(eps_t[:], smc_s[:], ctx_s[0][0:1, :].bitcast(F32))
                # ctx chunk0 row 0 := ratio*eps*sumctx (consumed by qp ones col)
                nc.scalar.mul(ctx_s[0][0:1, :], eps_t[:], RATIO * EPS)

                if LIMIT == "k2":
                    continue
                # Q pass
                qb = big.tile([128, NT, DH], F32, tag="hb", bufs=3, name="qb")
                nc.sync.dma_start(
                    qb[:], qkv_d.ap()[0, h].rearrange("(j p) e -> p j e", p=128))
                for j in range(NT):
                    diagq = sml.tile([128, 1], F32, tag="diagq")
                    sqj = strm.tile([128, DH], F32, tag="sqj")
                    nc.scalar.activation(sqj[:], qb[:, j, :], AF.Square,
                                         accum_out=diagq[:])
                    qtp = pst.tile([DH, 128], F32, tag="tp64")
                    nc.tensor.transpose(qtp[:], qb[:, j, :], ident_f[:])
                    qt = strm.tile([DH, 128], F32R, tag="kt")
                    nc.scalar.copy(qt[:], qtp[:])
                    dash = ps.tile([128, M], F32, tag="b256")
                    nc.tensor.matmul(dash[:], qt[:], projc_r[:], start=True, stop=True)
                    rmaxq = sml.tile([128, 1], F32, tag="rmaxq")
                    nc.vector.reduce_max(rmaxq[:], dash[:], axis=AX)
                    biasq = sml.tile([128, 1], F32, tag="biasq")
                    nc.vector.tensor_scalar(biasq[:], diagq[:],
                                            -0.5 * CNORM * CNORM, LNR,
                                            op0=mybir.AluOpType.mult,
                                            op1=mybir.AluOpType.add)
                    nc.vector.tensor_sub(biasq[:], biasq[:], rmaxq[:])
                    qp = strm.tile([128, M + 1], F32R, tag="qp")
                    nc.scalar.activation(qp[:, 1:M+1], dash[:], AF.Exp,
                                         bias=biasq[:], scale=1.0)
                    nc.scalar.activation(qp[:, 0:1], qp[:, 1:2], AF.Identity,
                                         bias=1.0, scale=0.0)
                    qpt_ps = ps.tile([128, 384], F32R, tag="qpt", bufs=1)
                    for mc, (off, w) in enumerate(MCH):
                        nc.tensor.transpose(qpt_ps[0:w, mc*128:mc*128+128],
                                            qp[:, off:off+w], ident_r[:])
                    qpt = strm.tile([128, 384], F32R, tag="qpts")
                    nc.vector.tensor_copy(qpt[:, 0:256], qpt_ps[:, 0:256])
                    nc.vector.tensor_copy(qpt[0:11, 256:384], qpt_ps[0:11, 256:384])
                    oe_ps = pst.tile([128, 66], F32, tag="oe")
                    for mc, (off, w) in enumerate(MCH):
                        nc.tensor.matmul(oe_ps[:], qpt[0:w, mc*128:mc*128+128],
                                         ctx_s[mc][:], start=(mc == 0), stop=(mc == 2))
                    oe = strm.tile([128, 66], F32, tag="oes")
                    nc.scalar.copy(oe[:], oe_ps[:])
                    dinv = sml.tile([128, 1], F32, tag="dinv")
                    nc.vector.reciprocal(dinv[:], oe[:, DH:DH+1])
                    osc = strm.tile([128, DH], F32R, tag="osc")
                    nc.vector.tensor_scalar_mul(osc[:], oe[:, 0:DH], dinv[:])
                    ot_ps = pst.tile([DH, 128], F32R, tag="tp64")
                    nc.tensor.transpose(ot_ps[:], osc[:], ident_r[:])
                    if h == 0 and j == 0:
                        otb = otp_pool.tile([128, 2, N], F32R, tag="otb")
                    pb = (h % 2) * 64
                    nc.scalar.copy(otb[pb:pb+DH, h // 2, j*128:(j+1)*128], ot_ps[:])

            # ---- phase 3: output projection ----
            if LIMIT != "all":
                for j in range(NT):
                    y_z = strm.tile([128, 1024], F32, tag="ys", bufs=2)
                    nc.vector.memset(y_z[:], 0.0)
                    nc.sync.dma_start(y.ap()[j*128:(j+1)*128, :], y_z[:])
            # Each matmul contracts a head PAIR (K=128): otb chunk ch stacks
            # heads 2ch (rows 0:64) and 2ch+1 (rows 64:128); wo_r stacks the
            # matching Wo rows.  All operands at base partition 0 — mixing
            # base partitions inside one PSUM accumulation group is fatal.
            for j in range(NT if LIMIT == "all" else 0):
                y_ps = [psc.tile([128, 512], F32, tag=f"ctx{nb}", name=f"yps{nb}") for nb in range(2)]
                for nb in range(2):
                    for ch in range(2):
                        nc.tensor.matmul(y_ps[nb][:],
                                         otb[:, ch, j*128:(j+1)*128],
                                         wo_r[:, ch*1024 + nb*512:
                                              ch*1024 + nb*512 + 512],
                                         start=(ch == 0), stop=(ch == 1))
                y_s = strm.tile([128, 1024], F32, tag="ys", bufs=2)
                for nb in range(2):
                    nc.vector.tensor_copy(y_s[:, nb*512:(nb+1)*512], y_ps[nb][:])
                nc.sync.dma_start(y.ap()[j*128:(j+1)*128, :], y_s[:])

    nc.compile()
    return nc


_prog = None


def _build_in_maps(inputs):
    return _make_in_maps(**inputs)


def _make_in_maps(x, Wq, Wk, Wv, Wo, bo, proj):
    x = np.asarray(x, np.float32)
    projc = np.ascontiguousarray(CNORM * np.asarray(proj, np.float32).T)
    identm = np.eye(128, dtype=np.float32)
    xTb = [np.ascontiguousarray(x[b].T) for b in range(B)]
    in_maps = []
    for c in range(8):
        b, g = c // 4, c % 4
        hs, he = g * 256, g * 256 + 256
        woT = np.asarray(Wo, np.float32)[:, hs:he].T          # [256, 1024]
        woP = np.concatenate([woT[:128], woT[128:]], axis=1)  # [128, 2048]
        in_maps.append({
            "xT": xTb[b],
            "wqT": np.ascontiguousarray(np.asarray(Wq, np.float32)[hs:he].T),
            "wkT": np.ascontiguousarray(np.asarray(Wk, np.float32)[hs:he].T),
            "wvT": np.ascontiguousarray(np.asarray(Wv, np.float32)[hs:he].T),
            "woP": np.ascontiguousarray(woP),
            "projc": projc,
            "ident": identm,
        })
    return in_maps


def kernel(x, Wq, Wk, Wv, Wo, bo, proj):
    global _prog
    if _prog is None:
        _prog = build()
    in_maps = _make_in_maps(x, Wq, Wk, Wv, Wo, bo, proj)
    res = run_bass_kernel_spmd(_prog, in_maps, core_ids=list(range(8)))
    out = np.zeros((B, N, D), np.float32)
    for c in range(8):
        out[c // 4] += res.results[c]["y"]
    out += np.asarray(bo, np.float32)[None, None, :]
    return out



# revision 12
# speedup vs baseline: 2.5077x; 2.5077x over previous
"""Performer (FAVOR+) linear attention on 8 TRN2 NeuronCores — v2.

Sharding: core c handles batch b=c//4 and head group g=c%4 (4 of 16 heads,
as 2 pairs).  Everything SBUF-resident (no DRAM scratch).

Per core:
  A.  Pair-transposed projections q2T/k2T/v2T [128(2h x 64), 2(pair), N]
      bf16, via fp32r matmuls with 512-wide moving operands.
  K.  Per pair/tile: dash_k = c*kT@projT; E = [1 | exp(dash_k)] raw (no
      bias: the per-row factor w_n = exp(-0.5c^2|k|^2) is folded into V',
      and the global e^{-mk} scale cancels in the output except through
      the eps terms, which are scaled by e^{+mk} instead).
      ctxT[65,267] += V'[128,65].T @ E accumulates in PSUM over tiles,
      V' = [w*v | w].  Finalize: mk = max dash (via max E); ctxT += eps *
      e^{mk} * [sv;N] per partition; Cx = ctxT.T in 3 chunks; Cx row 0 :=
      eps * colsum(Cx real rows)  (consumed by qp's ones column).
  Q.  Per tile: dash_q, mq = rowmax, diag via transpose+square+reduce,
      qp = [1 | exp(dash - 0.5c^2 diag - mq)] bf16, transpose, oe =
      qpT.T @ Cx (64 out cols + denominator col), divide, transpose into
      otb.
  P3. y_tile = otb.T @ Wo-pack, streamed out per tile.

All matmuls are bf16 x bf16 with fp32 PSUM accumulation.
"""
import sys
sys.path.insert(0, '/opt/trn_rl_repo')

import numpy as np
import ml_dtypes
import concourse.bass as bass
import concourse.bacc as bacc
import concourse.tile as tile
from concourse import mybir
from concourse.bass_utils import run_bass_kernel_spmd

F32 = mybir.dt.float32
F32R = mybir.dt.float32r
BF16 = mybir.dt.bfloat16
AX = mybir.AxisListType.X
AF = mybir.ActivationFunctionType
OP = mybir.AluOpType

B, N, D = 2, 4096, 1024
H, DH, M = 16, 64, 266
NT = N // 128
NC = N // 512
CN = DH ** -0.25
EPS = 1e-4
MCH = [(0, 128), (128, 128), (256, 11)]   # chunks over the 267-wide qp/E
LIMIT = "all"


def build():
    nc = bacc.Bacc("TRN2", target_bir_lowering=False, debug=False)

    xT = nc.dram_tensor("xT", [D, N], BF16, kind="ExternalInput")
    wP = nc.dram_tensor("wP", [128, 3, 2, 8, 128], BF16, kind="ExternalInput")
    woPb = nc.dram_tensor("woPb", [128, 2048], BF16, kind="ExternalInput")
    projc2 = nc.dram_tensor("projc2", [128, M], BF16, kind="ExternalInput")
    identB = nc.dram_tensor("identB", [128, 128], BF16, kind="ExternalInput")
    identF = nc.dram_tensor("identF", [128, 128], F32, kind="ExternalInput")
    svN = nc.dram_tensor("svN", [65, 4], F32, kind="ExternalInput")
    y = nc.dram_tensor("y", [N, D], F32, kind="ExternalOutput")

    with tile.TileContext(nc) as tc:
        with tc.tile_pool(name="const", bufs=1) as cpool, \
             tc.tile_pool(name="big", bufs=1) as big, \
             tc.tile_pool(name="xt", bufs=2) as xtp, \
             tc.tile_pool(name="strm", bufs=4) as strm, \
             tc.tile_pool(name="sml", bufs=4) as sml, \
             tc.tile_pool(name="psA", bufs=2, space="PSUM") as psA, \
             tc.tile_pool(name="psDa", bufs=2, space="PSUM") as psDa, \
             tc.tile_pool(name="psCtx", bufs=1, space="PSUM") as psCtx, \
             tc.tile_pool(name="psS", bufs=2, space="PSUM") as psS:

            # ---- constants ----
            wPs = cpool.tile([128, 3, 2, 8, 128], BF16, tag="wP")
            nc.sync.dma_start(wPs[:], wP.ap())
            woS = cpool.tile([128, 2048], BF16, tag="wo")
            nc.sync.dma_start(woS[:], woPb.ap())
            pjS = cpool.tile([128, M], BF16, tag="pj")
            nc.sync.dma_start(pjS[:], projc2.ap())
            idB = cpool.tile([128, 128], BF16, tag="idB")
            nc.sync.dma_start(idB[:], identB.ap())
            idF = cpool.tile([128, 128], F32, tag="idF")
            nc.sync.dma_start(idF[:], identF.ap())
            svS = cpool.tile([65, 4], F32, tag="sv")
            nc.sync.dma_start(svS[:], svN.ap())

            # ---- persistent tensors ----
            q2T = big.tile([128, 2, N], BF16, tag="q2T")
            k2T = big.tile([128, 2, N], BF16, tag="k2T")
            v2T = big.tile([128, 2, N], BF16, tag="v2T")
            otb = big.tile([128, 2, N], BF16, tag="otb")
            Eb = big.tile([128, 2, NT, M + 1], BF16, tag="Eb")
            Cx = [big.tile([128, 3, 65], BF16, tag=f"cx{p}", name=f"cx{p}")
                  for p in range(2)]

            nc.vector.memset(Eb[:, :, :, 0:1], 1.0)  # ones column for k_cumsum

            qkv_dst = [q2T, k2T, v2T]

            def copy3(t, dst, src):
                if t == 1:
                    nc.scalar.copy(dst, src)
                else:
                    nc.vector.tensor_copy(dst, src)

            def phase_a(cc):
                xt = xtp.tile([128, 8, 512], BF16, tag="xt")
                nc.sync.dma_start(xt[:], xT.ap().rearrange(
                    "(c p) n -> p c n", p=128)[:, :, cc*512:(cc+1)*512])
                for t in range(3):
                    for pr in range(2):
                        acc = psA.tile([128, 512], F32, tag="a512")
                        for dch in range(8):
                            nc.tensor.matmul(acc[:],
                                             wPs[:, t, pr, dch, :],
                                             xt[:, dch, :],
                                             start=(dch == 0), stop=(dch == 7))
                        copy3(t, qkv_dst[t][:, pr, cc*512:(cc+1)*512], acc[:])

            def k_tile(pr, j, ctx_ps, scr):
                kn = scr[:, 128:256]                    # [128,128] bf16
                vn = scr[:, 256:384]
                for hh in range(2):
                    pb = hh * 64
                    dk = psDa.tile([128, M], F32, tag="dash")
                    nc.tensor.matmul(dk[:], k2T[pb:pb+64, pr, j*128:(j+1)*128],
                                     pjS[pb:pb+64, :], start=True, stop=True)
                    nc.scalar.activation(Eb[:, hh, j, 1:M+1], dk[:], AF.Exp)
                nc.tensor.transpose(kn, k2T[:, pr, j*128:(j+1)*128], idB[:])
                nc.tensor.transpose(vn, v2T[:, pr, j*128:(j+1)*128], idB[:])
                sq = strm.tile([128, 128], BF16, tag="sq")
                nc.scalar.activation(sq[:], kn, AF.Square)
                dg2 = sml.tile([128, 2], F32, tag="dg2")
                nc.vector.reduce_sum(dg2[:], sq[:].rearrange("p (h e) -> p h e", e=64),
                                     axis=AX)
                w2 = sml.tile([128, 2], F32, tag="w2")
                nc.scalar.activation(w2[:], dg2[:], AF.Exp, scale=-0.5 * CN * CN)
                for hh in range(2):
                    vt = strm.tile([128, 65], BF16, tag="vt")
                    nc.vector.tensor_scalar_mul(vt[:, 0:64], vn[:, hh*64:hh*64+64],
                                                w2[:, hh:hh+1])
                    nc.vector.tensor_copy(vt[:, 64:65], w2[:, hh:hh+1])
                    nc.tensor.matmul(ctx_ps[hh][:], vt[:], Eb[:, hh, j, :],
                                     start=(j == 0), stop=(j == NT - 1))

            def k_finalize(pr, ctx_ps):
                scr = psS.tile([128, 1024], BF16, tag="scr")
                for hh in range(2):
                    h = pr * 2 + hh
                    m1 = sml.tile([128, NT], BF16, tag="m1")
                    nc.vector.reduce_max(m1[:], Eb[:, hh, :, 1:M+1], axis=AX)
                    m2 = sml.tile([128, 1], BF16, tag="m2")
                    nc.vector.reduce_max(m2[:], m1[:], axis=AX)
                    mrow = scr[0:1, 128:256]
                    nc.tensor.transpose(mrow, m2[:], idB[:])
                    emk = sml.tile([1, 1], F32, tag="emk")
                    nc.vector.reduce_max(emk[:], mrow, axis=AX)
                    emkb = sml.tile([65, 1], F32, tag="emkb")
                    nc.gpsimd.partition_broadcast(emkb[:], emk[:])
                    epscol = sml.tile([65, 1], F32, tag="epscol")
                    nc.vector.tensor_mul(epscol[:], emkb[:], svS[:, h:h+1])
                    cts = strm.tile([65, M + 1], BF16, tag="cts")
                    nc.vector.tensor_scalar_add(cts[:], ctx_ps[hh][:], epscol[:])
                    scc = sml.tile([65, 1], F32, tag="scc")
                    nc.vector.reduce_sum(scc[:], cts[:, 1:M+1], axis=AX)
                    scrow = scr[0:1, 0:130].bitcast(F32)        # [1, 65] f32
                    nc.tensor.transpose(scrow, scc[:], idF[0:65, 0:65])
                    for mc, (off, wd) in enumerate(MCH):
                        cxp = scr[0:128, 384:449]               # [128, 65] bf16
                        nc.tensor.transpose(cxp[0:wd, :], cts[:, off:off+wd],
                                            idB[0:65, 0:65])
                        nc.vector.tensor_copy(Cx[hh][0:wd, mc, :], cxp[0:wd, :])
                    nc.scalar.mul(Cx[hh][0:1, 0, :], scrow, EPS)

            def q_tile(pr, j, scr):
                qn = scr[:, 0:128]
                nc.tensor.transpose(qn, q2T[:, pr, j*128:(j+1)*128], idB[:])
                sq = strm.tile([128, 128], BF16, tag="sq")
                nc.scalar.activation(sq[:], qn, AF.Square)
                dg2 = sml.tile([128, 2], F32, tag="dg2")
                nc.vector.reduce_sum(dg2[:], sq[:].rearrange("p (h e) -> p h e", e=64),
                                     axis=AX)
                for hh in range(2):
                    pb = hh * 64
                    dq = psDa.tile([128, M], F32, tag="dash")
                    nc.tensor.matmul(dq[:], q2T[pb:pb+64, pr, j*128:(j+1)*128],
                                     pjS[pb:pb+64, :], start=True, stop=True)
                    rmax = sml.tile([128, 1], F32, tag="rmax")
                    nc.vector.reduce_max(rmax[:], dq[:], axis=AX)
                    bias = sml.tile([128, 1], F32, tag="bias")
                    nc.vector.tensor_scalar(bias[:], dg2[:, hh:hh+1],
                                            -0.5 * CN * CN, rmax[:],
                                            op0=OP.mult, op1=OP.subtract)
                    qp = strm.tile([128, M + 1], BF16, tag="qp")
                    nc.scalar.activation(qp[:, 1:M+1], dq[:], AF.Exp,
                                         bias=bias[:], scale=1.0)
                    nc.scalar.activation(qp[:, 0:1], dq[:, 0:1], AF.Identity,
                                         bias=1.0, scale=0.0)
                    # qpt/oe regions are shared between the two heads (WAR
                    # deps serialize them; cross-tile overlap via the ring)
                    qpt_ps = scr[:, 384:768].rearrange("p (c n) -> p c n", n=128)
                    for mc, (off, wd) in enumerate(MCH):
                        nc.tensor.transpose(qpt_ps[0:wd, mc, :],
                                            qp[:, off:off+wd], idB[:])
                    qpt = strm.tile([128, 3, 128], BF16, tag="qpts")
                    if hh == 0:
                        nc.vector.tensor_copy(qpt[:], qpt_ps)
                    else:
                        nc.scalar.copy(qpt[:], qpt_ps)
                    oe = scr[:, 768:898].bitcast(F32)            # [128, 65]
                    for mc, (off, wd) in enumerate(MCH):
                        nc.tensor.matmul(oe, qpt[0:wd, mc, :], Cx[hh][0:wd, mc, :],
                                         start=(mc == 0), stop=(mc == 2))
                    dinv = sml.tile([128, 1], F32, tag="dinv")
                    nc.vector.reciprocal(dinv[:], oe[:, 64:65])
                    osc = strm.tile([128, 64], BF16, tag="osc")
                    nc.vector.tensor_scalar_mul(osc[:], oe[:, 0:64], dinv[:])
                    ot = scr[pb:pb+64, 0:128]                   # reuse qn slice
                    nc.tensor.transpose(ot, osc[:], idB[:])
                    if hh == 0:
                        nc.vector.tensor_copy(otb[pb:pb+64, pr, j*128:(j+1)*128], ot)
                    else:
                        nc.scalar.copy(otb[pb:pb+64, pr, j*128:(j+1)*128], ot)

            def p3_tile(j):
                ys = strm.tile([128, 1024], F32, tag="ys")
                for half in range(2):
                    yp = psA.tile([128, 512], F32, tag="a512")
                    for pr in range(2):
                        nc.tensor.matmul(yp[:], otb[:, pr, j*128:(j+1)*128],
                                         woS[:, pr*1024 + half*512:
                                             pr*1024 + half*512 + 512],
                                         start=(pr == 0), stop=(pr == 1))
                    if half == 0:
                        nc.vector.tensor_copy(ys[:, 0:512], yp[:])
                    else:
                        nc.scalar.copy(ys[:, 512:1024], yp[:])
                nc.sync.dma_start(y.ap()[j*128:(j+1)*128, :], ys[:])

            def zero_y():
                zs = strm.tile([128, 1024], F32, tag="ys")
                nc.vector.memset(zs[:], 0.0)
                for j in range(NT):
                    nc.sync.dma_start(y.ap()[j*128:(j+1)*128, :], zs[:])

            # ---------- schedule ----------
            ctx0 = [psCtx.tile([65, M + 1], F32, tag=f"ctx{hh}", name=f"c0_{hh}")
                    for hh in range(2)]
            for cc in range(NC):
                phase_a(cc)
                if LIMIT != "a":
                    for j in range(cc*4, cc*4 + 4):
                        scr = psS.tile([128, 1024], BF16, tag="scr")
                        k_tile(0, j, ctx0, scr)
            if LIMIT == "a":
                zero_y()
            else:
                k_finalize(0, ctx0)
                ctx1 = [psCtx.tile([65, M + 1], F32, tag=f"ctx{hh}", name=f"c1_{hh}")
                        for hh in range(2)]
                for j in range(NT):
                    scr = psS.tile([128, 1024], BF16, tag="scr")
                    q_tile(0, j, scr)
                    if LIMIT != "k0":
                        k_tile(1, j, ctx1, scr)
                if LIMIT == "k0":
                    zero_y()
                else:
                    k_finalize(1, ctx1)
                    for j in range(NT):
                        scr = psS.tile([128, 1024], BF16, tag="scr")
                        q_tile(1, j, scr)
                        p3_tile(j)

    nc.compile()
    return nc


_prog = None


def _build_in_maps(inputs):
    return _make_in_maps(**inputs)


def _make_in_maps(x, Wq, Wk, Wv, Wo, bo, proj):
    x = np.asarray(x, np.float32)
    Wq = np.asarray(Wq, np.float32)
    Wk = np.asarray(Wk, np.float32)
    Wv = np.asarray(Wv, np.float32)
    Wo = np.asarray(Wo, np.float32)
    proj = np.asarray(proj, np.float32)
    cp = np.ascontiguousarray(CN * proj.T)                    # [64, 266]
    projc2 = np.concatenate([cp, cp], axis=0).astype(ml_dtypes.bfloat16)
    identB = np.eye(128, dtype=ml_dtypes.bfloat16)
    identF = np.eye(128, dtype=np.float32)
    xTb = [np.ascontiguousarray(x[b].T) for b in range(B)]
    xsum = [x[b].sum(axis=0) for b in range(B)]               # [1024]
    in_maps = []
    for c in range(8):
        b, g = c // 4, c % 4
        rows = slice(g * 256, g * 256 + 256)
        wPm = np.empty([128, 3, 2, 8, 128], np.float32)
        for t, W in enumerate((Wq, Wk, Wv)):
            blk = W[rows]                                     # [256, 1024]
            for pr in range(2):
                wPm[:, t, pr] = (blk[pr*128:(pr+1)*128].T
                                 .reshape(8, 128, 128).transpose(1, 0, 2))
        woT = Wo[:, rows].T                                   # [256, 1024]
        woP = np.concatenate([woT[:128], woT[128:]], axis=1)  # [128, 2048]
        svNm = np.empty([65, 4], np.float32)
        for h in range(4):
            wvh = Wv[g*256 + h*64: g*256 + (h+1)*64]          # [64, 1024]
            svNm[0:64, h] = EPS * (wvh @ xsum[b])
            svNm[64, h] = EPS * N
        in_maps.append({
            "xT": xTb[b].astype(ml_dtypes.bfloat16),
            "wP": np.ascontiguousarray(wPm).astype(ml_dtypes.bfloat16),
            "woPb": np.ascontiguousarray(woP).astype(ml_dtypes.bfloat16),
            "projc2": projc2,
            "identB": identB,
            "identF": identF,
            "svN": svNm,
        })
    return in_maps


def kernel(x, Wq, Wk, Wv, Wo, bo, proj):
    global _prog
    if _prog is None:
        _prog = build()
    in_maps = _make_in_maps(x, Wq, Wk, Wv, Wo, bo, proj)
    res = run_bass_kernel_spmd(_prog, in_maps, core_ids=list(range(8)))
    out = np.zeros((B, N, D), np.float32)
    for c in range(8):
        out[c // 4] += res.results[c]["y"]
    out += np.asarray(bo, np.float32)[None, None, :]
    return out
